# revision 26
# baseline (speedup 1.0000x reference)
"""Self-attention kernel for Trainium2, 8 NeuronCores, data-parallel over batch.

Reference computation (per batch sample, N=H*W=4096, C=64, Ck=8):
    f = x @ Wf + bf            [N, 8]
    g = x @ Wg + bg            [N, 8]
    h = x @ Wh + bh            [N, 64]
    s = f @ g^T                [N, N]
    attn = softmax(s, axis=-1)
    o = gamma * (attn @ h) + x

Kernel strategy (one sample per core):
  - Scores computed TRANSPOSED: sT[m, n] with m (the softmax-reduction index)
    on partitions.  The contraction dim is only K=9 (8 features + affine
    row), so four m-tiles' score matmuls run CONCURRENTLY in the four 32-row
    PE tile_position row groups (f/g both band-replicated across the 4
    bands).  No max subtraction (scores are O(1)); the softmax denominator
    comes free from an augmented column in h.
  - exp split across ScalarE (true exp via activation affine) and VectorE
    (fp8e4m3 Schraudolph bit-trick: i8 = max(s'/16, 0) bitcast to e4m3
    = exp(s)/8), Bresenham-interleaved over [128, 1024] PSUM chunks.
    Scores carry a C1=128*log2(e) scale and +504 offset folded into the
    weights.
  - ctx^T = [128*gamma*h | 128]^T @ exp accumulated in PSUM over m with
    fp8 DoubleRow matmuls (two m-tiles per instruction); row 64 gives
    128*sum(exp), whose reciprocal directly yields gamma*ctx.
  - Epilogue: DMA-transpose ctxT back to [n, c] layout (PE transposes for
    the final quarter), one batched reciprocal per quarter on the DVE, then
    a single fused (ctx*rden + x) scalar_tensor_tensor per n-tile on
    GpSimd so the exp engines stay dedicated to exp.
"""

import numpy as np
import ml_dtypes

import concourse.bass as bass
import concourse.mybir as mybir
import concourse.tile as tile
from concourse.bass import ts, ds
from concourse.bass_utils import run_bass_kernel_spmd
from concourse.masks import make_identity

BF16 = mybir.dt.bfloat16
FP8 = mybir.dt.float8e4
F32 = mybir.dt.float32

N = 4096          # H*W per sample
C = 64            # channels
CK = 8            # f/g projection dim
P = 128           # partitions
NT = N // P       # 32 n/m tiles
HALF = N // 2     # 2048
QW = 1024         # quarter width
NQ = N // QW      # 4
QT = QW // P      # 8 n-tiles per quarter
C1 = 128.0 * np.log2(np.e)   # score pre-scale (f side), undone by ACT affine
SCORE_OFF = 504.0            # additive score offset (exact in bf16):
                             # s'/16 = 8*log2(e)*s + 31.5, the e4m3 bit value
                             # of exp(s)/8 with the Schraudolph shift; clamped
                             # at 0 on the DVE.  ACT computes exp(s - ln 8).
                             # The /8 keeps exp in e4m3 range; softmax ratios
                             # are unaffected.

def _np_bf16(a):
    return np.ascontiguousarray(a.astype(np.float32).astype(ml_dtypes.bfloat16))


def prepare_weights(Wf, bf, Wg, bg, Wh, bh, gamma):
    """Host-side weight folding. Returns dict of bf16 arrays (dram params)."""
    Wf = np.asarray(Wf, np.float32)
    Wg = np.asarray(Wg, np.float32)
    Wh = np.asarray(Wh, np.float32)
    bf = np.asarray(bf, np.float32)
    bg = np.asarray(bg, np.float32)
    bh = np.asarray(bh, np.float32)
    gamma = float(np.asarray(gamma, np.float32))

    # f-side, scaled by C1, bias as row 8 of each band; replicated across
    # the 4 32-row bands so the 4-way row-group-packed score matmuls can
    # contract per band.  Column 32b+8 (paired with g-side column 8 == 1)
    # adds SCORE_OFF to every score so the DVE fp8 bit-trick can clamp at 0
    # instead of going negative: raw scores' = C1*s + SCORE_OFF.
    wf_aug = np.zeros((128, 128), np.float32)
    for b in range(4):
        wf_aug[:C, 32 * b: 32 * b + CK] = C1 * Wf
        wf_aug[C, 32 * b: 32 * b + CK] = C1 * bf
        wf_aug[C, 32 * b + CK] = SCORE_OFF

    # g-side, same replication, column 32b+8 = ones row
    wg_aug = np.zeros((128, 128), np.float32)
    for b in range(4):
        wg_aug[:C, 32 * b: 32 * b + CK] = Wg
        wg_aug[C, 32 * b: 32 * b + CK] = bg
        wg_aug[C, 32 * b + CK] = 1.0

    # h-side scaled by 128*gamma (keeps fp8 h out of subnormals) and a
    # 128-valued denominator column at 64; the epilogue's reciprocal of
    # 128*sum(exp) then yields gamma*ctx directly: [128, 128]
    wh_aug = np.zeros((128, 128), np.float32)
    wh_aug[:C, :C] = 128.0 * gamma * Wh
    wh_aug[C, :C] = 128.0 * gamma * bh
    wh_aug[C, C] = 128.0

    return {
        "wf": _np_bf16(wf_aug),
        "wg": _np_bf16(wg_aug),
        "wh": _np_bf16(wh_aug),
    }


def _spill_excess_waits(nc, limit=1):
    """Walrus rejects HW-queue instructions carrying more than a couple of
    semaphore waits.  Move excess waits onto standalone EventSemaphore
    instructions inserted just before the offender on the same engine
    (cumulative sem-ge waits split across instructions are equivalent)."""
    n_spill = 0
    for bb in nc.main_func.blocks:
        rebuilt = []
        changed = False
        for ins in bb.instructions:
            si = ins.sync_info
            if si is not None and len(si.on_wait) > limit:
                waits = list(si.on_wait)
                for w in waits[limit:]:
                    ev = mybir.InstEventSemaphore(
                        name=f"wspill-{n_spill}", ins=[], outs=[])
                    ev.engine = ins.engine
                    ev.sync_info = mybir.SyncInfo(on_wait=[w], on_update=[])
                    rebuilt.append(ev)
                    n_spill += 1
                ins.sync_info = mybir.SyncInfo(
                    on_wait=waits[:limit], on_update=list(si.on_update))
                changed = True
            rebuilt.append(ins)
        if changed:
            bb.instructions = rebuilt
    return n_spill


def _dedup_ldweights(nc):
    """Drop an InstLdweights whose weight AP/mode is identical to the
    immediately preceding LDW on the PE queue (score j-chunk and DoubleRow
    pairs reuse the same stationary operand).  Only sync-free LDWs are
    dropped so no semaphore edges are lost."""
    n_drop = 0
    for bb in nc.main_func.blocks:
        rebuilt = []
        last_key = None
        changed = False
        for ins in bb.instructions:
            tname = type(ins).__name__
            if tname == "InstLdweights":
                si = ins.sync_info
                clean = si is None or (not si.on_wait and not si.on_update)
                key = (str(ins.ins[0]), str(getattr(ins, "perf_mode", None)),
                       str(getattr(ins, "tile_position", None)),
                       str(getattr(ins, "is_transpose", None)))
                if clean and key == last_key:
                    n_drop += 1
                    changed = True
                    continue
                last_key = key
            elif tname == "InstMatmult":
                pass  # matmul leaves the stationary operand in place
            elif ins.engine == mybir.EngineType.PE:
                last_key = None
            rebuilt.append(ins)
        if changed:
            bb.instructions = rebuilt
    return n_drop


def build_bass(repeat=1, spill=True):
    """Build the per-core Bass graph (SPMD: same graph on all 8 cores).
    repeat > 1 duplicates the whole body for timing calibration."""
    nc = bass.Bass()

    x_d = nc.declare_dram_parameter("x", [P, NT * C], F32, isOutput=False)
    wf_d = nc.declare_dram_parameter("wf", [128, 128], BF16, isOutput=False)
    wg_d = nc.declare_dram_parameter("wg", [128, 128], BF16, isOutput=False)
    wh_d = nc.declare_dram_parameter("wh", [128, 128], BF16, isOutput=False)
    xt_d = nc.declare_dram_parameter("xta", [65, N], BF16, isOutput=False)
    out_d = nc.declare_dram_parameter("out", [N, C], F32, isOutput=True)

    with tile.TileContext(nc) as tc:
        for _ in range(repeat):
            _build_body(nc, tc, x_d, wf_d, wg_d, wh_d, xt_d, out_d)
    _dedup_ldweights(nc)
    if spill:
        _spill_excess_waits(nc)
    return nc


def _build_body(nc, tc, x_d, wf_d, wg_d, wh_d, xt_d, out_d):
    from contextlib import ExitStack

    with ExitStack() as ctx:
        consts = ctx.enter_context(tc.tile_pool(name="consts", bufs=1))
        exp_pool = ctx.enter_context(tc.tile_pool(name="expp", bufs=3))
        work = ctx.enter_context(tc.tile_pool(name="work", bufs=6))

        # ---- prologue DMA: xta + weights first (they gate the projections),
        # residual x tiles behind them.  sync (HWDGE) + gpsimd (SWDGE) queues
        # only: scalar stays free for exp ----
        x_sb = consts.tile([P, NT, C], F32)
        x3 = x_d.rearrange("p (t c) -> p t c", c=C)
        wf_sb = consts.tile([128, 128], BF16)
        wg_sb = consts.tile([128, 128], BF16)
        wh_sb = consts.tile([128, 128], BF16)
        nc.sync.dma_start(wf_sb[:], wf_d[:])
        nc.gpsimd.dma_start(wg_sb[:], wg_d[:])
        nc.sync.dma_start(wh_sb[:], wh_d[:])

        # identity for the final-quarter PE transposes (PE is idle then)
        id_sb = consts.tile([128, 128], BF16)
        make_identity(nc, id_sb[:])

        # --- head warmup: engines are otherwise idle for the NEFF startup +
        # input DMA.  Pull the ScalarE exp table load (~2.7us) and the PE HAM
        # un-throttle (~3.4us of sustained activity) into that window.
        warm = consts.tile([128, 512], BF16)
        nc.vector.memset(warm[:], 0.0)
        wtmp = consts.tile([128, 8], BF16)
        nc.scalar.activation(wtmp[:], warm[:, :8],
                             mybir.ActivationFunctionType.Exp,
                             bias=0.0, scale=1.0)
        with tc.tile_pool(name="warm_ps", bufs=1, space="PSUM") as warm_ps:
            wp = warm_ps.tile([128, 512], F32)
            for _ in range(20):
                nc.tensor.matmul(wp[:], warm[:, :128], warm[:],
                                 start=True, stop=True)

        # ACT exp bias: exp(s'/C1 + bias) = exp(s - ln 8)
        ebias = consts.tile([P, 1], F32)
        nc.vector.memset(ebias[:], float(-SCORE_OFF / C1 - np.log(8.0)))

        # ---- xT_aug [128, N] bf16: rows 0..64 host-built [x^T ; ones],
        # rows 65..127 zeroed on device ----
        xt_sb = consts.tile([128, N], BF16)
        nc.vector.memset(xt_sb[C:, :], 0.0)
        for d in range(2):
            (nc.sync if d == 0 else nc.gpsimd).dma_start(
                xt_sb[:65, ds(d * HALF, HALF)], xt_d[:, ds(d * HALF, HALF)])
        # residual x tiles 0..15 (quarters 0-1), behind xta on the queues
        for d in range(4):
            (nc.sync if d % 2 == 0 else nc.gpsimd).dma_start(
                x_sb[:, ds(4 * d, 4), :], x3[:, ds(4 * d, 4), :])

        with tc.tile_pool(name="pro_ps", bufs=3, space="PSUM") as pro_ps:
            # f/g projections (f scaled by C1), band-replicated.  Emission
            # order front-loads exactly what main-loop group (q0, t) needs:
            # f chunks 0-1 (n 0:1024), then per-t g chunk + h group.
            f_sb = consts.tile([128, N], BF16)
            g_sb = consts.tile([128, N], BF16)
            h_sb = consts.tile([P, NT, 128], FP8)

            def emit_f(chunk):
                pf = pro_ps.tile([128, 512], F32, tag="fg", name="pf")
                nc.tensor.matmul(pf[:], wf_sb[:, :], xt_sb[:, ts(chunk, 512)],
                                 start=True, stop=True)
                nc.any.tensor_copy(f_sb[:, ts(chunk, 512)], pf[:])

            def emit_g(chunk):
                pg = pro_ps.tile([128, 512], F32, tag="fg", name="pg")
                nc.tensor.matmul(pg[:], wg_sb[:, :], xt_sb[:, ts(chunk, 512)],
                                 start=True, stop=True)
                nc.any.tensor_copy(g_sb[:, ts(chunk, 512)], pg[:])

            def emit_h(grp):
                ph = pro_ps.tile([128, 512], F32, tag="fg", name="ph")
                for j in range(4):
                    m = 4 * grp + j
                    nc.tensor.matmul(ph[:, ts(j, P)], xt_sb[:, ts(m, P)],
                                     wh_sb[:], start=True, stop=True)
                nc.any.tensor_copy(h_sb[:, ds(4 * grp, 4), :], ph[:])

            emit_f(0)
            emit_f(1)
            for grp in range(8):
                emit_g(grp)
                emit_h(grp)
            for chunk in range(2, 8):
                emit_f(chunk)

        # x tiles 16..31 (quarters 2-3 residuals, needed late): behind
        # the compute-critical loads on each queue
        for d in range(4, 8):
            (nc.sync if d % 2 == 0 else nc.gpsimd).dma_start(
                x_sb[:, ds(4 * d, 4), :], x3[:, ds(4 * d, 4), :])

        # ---- main: scores -> exp -> ctxT accumulate; epilogue, per quarter.
        # Score matmuls are 4-way row-group packed: band b (rows 32b..32b+31)
        # computes m-tile 4t+b.  Each (q, t) group produces four [128, 2, 512]
        # PSUM pair-tiles (2 banks each; pool of 2 + ctx 2 banks + 1 keep-warm
        # bank = 7 banks).
        #
        # HAM keep-warm: row-group and DoubleRow matmuls do NOT register as
        # PE activity for the HAM clock gate (hardware-measured: the PE
        # re-throttles to 1.2 GHz once only packed/DR matmuls remain), so one
        # small full-array bf16 matmul per group feeds the activity monitor.
        # Its output is allocated from the score pool so pool rotation anchors
        # it to the group cadence (a dep-free dummy gets hoisted to the start
        # by the Tile scheduler).
        with tc.tile_pool(name="ps_s", bufs=2, space="PSUM") as ps_s, \
             tc.tile_pool(name="ps_tr", bufs=2, space="PSUM") as ps_tr, \
             tc.tile_pool(name="ps_ctx", bufs=1, space="PSUM") as ps_ctx:
            # exp engine assignment: ACT chunk ~997ns vs DVE ~1192ns -> give
            # ACT ~70 of 128 chunks, spread evenly (Bresenham).
            N_CHUNKS = 128
            DVE_SHARE = 54
            use_dve = [((i * DVE_SHARE) % N_CHUNKS) < DVE_SHARE
                       for i in range(N_CHUNKS)]
            chunk_idx = 0

            def emit_epi_head(q, ctx_ps):
                # epilogue head for quarter q: copy ctxT to SBUF bf16 (halves
                # so transposes start before the full copy) and transpose the
                # 8 [128, 128] blocks back to [n, c] ON THE PE (in-stream;
                # DMA-xbar transposes serialized ~1.2us each on an engine
                # queue and their sem chain stalled the exp engines >3.4us
                # per quarter boundary, re-throttling the HAM gate).
                # Emitted DEFERRED, inside the next quarter's first group.
                ctxt_sb = work.tile([128, QW], BF16, tag="ctxt")
                nc.any.tensor_copy(ctxt_sb[:, :QW // 2], ctx_ps[:, :QW // 2])
                nc.any.tensor_copy(ctxt_sb[:, QW // 2:], ctx_ps[:, QW // 2:])
                tr_ps = ps_tr.tile([128, QW], BF16, tag="tr", name="trps")
                for t2 in range(QT):
                    nc.tensor.transpose(tr_ps[:, ts(t2, P)],
                                        ctxt_sb[:, ts(t2, P)], id_sb[:])
                return tr_ps.rearrange("p (t c) -> p t c", c=P)

            def emit_epi_tile(q, tr3, t2):
                # one n-tile of quarter q's epilogue: reciprocal of the
                # denominator column, fused (ctx*rden + x), store.  Spread
                # one-per-group across the next quarter.
                blk = tr3[:, t2, :]
                rden = work.tile([P, 1], F32, tag="rden")
                nc.vector.reciprocal(rden[:], blk[:, C: C + 1])
                osb = work.tile([P, C], F32, tag="osb")
                nc.vector.scalar_tensor_tensor(
                    osb[:], blk[:, :C], rden[:],
                    x_sb[:, q * QT + t2, :],
                    mybir.AluOpType.mult, mybir.AluOpType.add)
                (nc.gpsimd if t2 % 2 == 0 else nc.sync).dma_start(
                    out_d[ds((q * QT + t2) * P, P), :], osb[:])

            prev_ctx = None
            prev_otr = None
            for q in range(NQ):
                ctx_ps = ps_ctx.tile([128, QW], F32, tag="ctx")
                for t in range(8):
                    e_q = exp_pool.tile([128, 4, QW], FP8, tag="e")
                    # Four [128, 2, 512] PSUM pair-tiles per group:
                    #   spF: m-tile 4t+0 via FULL-K matmuls (j0|j1).  The
                    #        band-replicated f/g make full-K contract all 4
                    #        band copies -> 4x the score, undone by a /4 in
                    #        this chunk's exp scale.  These full-array
                    #        matmuls keep the HAM clock gate warm (packed/DR
                    #        matmuls are invisible to it).
                    #   spA/spB: m-tiles 4t+1, 4t+2 row-group packed (bands
                    #        1, 2), one tile per j-chunk.
                    #   spC: m-tile 4t+3 (band 3), j0|j1.
                    spF = ps_s.tile([128, 2, 512], F32, tag="s", name="spF")
                    spA = ps_s.tile([128, 2, 512], F32, tag="s", name="spA")
                    spB = ps_s.tile([128, 2, 512], F32, tag="s", name="spB")
                    spC = ps_s.tile([128, 2, 512], F32, tag="s", name="spC")
                    m0 = 4 * t
                    for j in range(2):
                        nc.tensor.matmul(
                            spF[:, j, :], g_sb[:, ts(m0, P)],
                            f_sb[:, ds(q * QW + j * 512, 512)],
                            start=True, stop=True)
                    for b in (1, 2):
                        for j in range(2):
                            spt = spA if j == 0 else spB
                            nc.tensor.matmul(
                                spt[:, b - 1, :],
                                g_sb[ds(32 * b, 32), ts(m0 + b, P)],
                                f_sb[ds(32 * b, 32), ds(q * QW + j * 512, 512)],
                                start=True, stop=True,
                                tile_position=(32 * b, 0))
                    for j in range(2):
                        nc.tensor.matmul(
                            spC[:, j, :],
                            g_sb[ds(96, 32), ts(m0 + 3, P)],
                            f_sb[ds(96, 32), ds(q * QW + j * 512, 512)],
                            start=True, stop=True,
                            tile_position=(96, 0))
                    # exp: 4 [128, 1024] chunks; spF carries 4x scores
                    for spt, dst, quad in (
                            (spF, e_q[:, ds(0, 1), :], True),
                            (spA, e_q[:, ds(1, 2), ds(0, 512)], False),
                            (spB, e_q[:, ds(1, 2), ds(512, 512)], False),
                            (spC, e_q[:, ds(3, 1), :], False)):
                        scl = 0.25 if quad else 1.0
                        if use_dve[chunk_idx]:
                            i8_view = e_q.bitcast(mybir.dt.int8)
                            i8_dst = (i8_view[:, ds(0, 1), :] if spt is spF
                                      else i8_view[:, ds(1, 2), ds(0, 512)] if spt is spA
                                      else i8_view[:, ds(1, 2), ds(512, 512)] if spt is spB
                                      else i8_view[:, ds(3, 1), :])
                            nc.vector.tensor_scalar(
                                i8_dst, spt[:], scl / 16.0, 0.0,
                                mybir.AluOpType.mult, mybir.AluOpType.max)
                        else:
                            nc.scalar.activation(
                                dst, spt[:],
                                mybir.ActivationFunctionType.Exp,
                                bias=ebias[:], scale=float(scl / C1))
                        chunk_idx += 1
                    # deferred epilogue of the previous quarter: head (copy +
                    # transposes) at t=0 (must precede this quarter's first
                    # ctx matmuls: ps_ctx has bufs=1, so the copies must be
                    # emitted before the bank reuse); one tile's tail per
                    # group thereafter
                    if prev_ctx is not None:
                        if t == 0:
                            prev_otr = emit_epi_head(q - 1, prev_ctx)
                        emit_epi_tile(q - 1, prev_otr, t)
                    # ctx accumulate: pair-major so each h pair's two j-chunk
                    # matmuls share one DoubleRow LDW (deduped)
                    for pr in range(2):
                        for j in range(2):
                            nc.tensor.matmul(
                                ctx_ps[:, ds(j * 512, 512)],
                                h_sb[:, ds(4 * t + 2 * pr, 2), :],
                                e_q[:, ds(2 * pr, 2), ds(j * 512, 512)],
                                perf_mode=mybir.MatmulPerfMode.DoubleRow,
                                start=(t == 0 and pr == 0),
                                stop=(t == 7 and pr == 1))
                prev_ctx = ctx_ps

            # tail: last quarter's epilogue (same PE-transpose path)
            tr3 = emit_epi_head(NQ - 1, prev_ctx)
            for t2 in range(QT):
                emit_epi_tile(NQ - 1, tr3, t2)


_CACHE = {}


def _get_nc():
    if "nc" not in _CACHE:
        _CACHE["nc"] = build_bass()
    return _CACHE["nc"]


def kernel(x, Wf, bf, Wg, bg, Wh, bh, gamma):
    x = np.asarray(x, np.float32)
    B = x.shape[0]
    assert x.shape == (B, 64, 64, 64) and B == 8

    w = prepare_weights(Wf, bf, Wg, bg, Wh, bh, gamma)
    nc = _get_nc()
    xt = x.reshape(B, NT, P, C).transpose(0, 2, 1, 3).reshape(B, P, NT * C)
    xta = np.ones((B, 65, N), np.float32)
    xta[:, :C, :] = x.reshape(B, N, C).transpose(0, 2, 1)
    xta = xta.astype(ml_dtypes.bfloat16)
    in_maps = [{"x": np.ascontiguousarray(xt[i]),
                "xta": np.ascontiguousarray(xta[i]), **w} for i in range(B)]
    res = run_bass_kernel_spmd(nc, in_maps, core_ids=list(range(8)))
    out = np.stack([np.asarray(res.results[i]["out"]).reshape(64, 64, 64)
                    for i in range(B)])
    return out.astype(np.float32)


# revision 30
# speedup vs baseline: 1.2758x; 1.2758x over previous
"""Self-attention kernel for Trainium2, 8 NeuronCores, data-parallel over batch.

Reference computation (per batch sample, N=H*W=4096, C=64, Ck=8):
    f = x @ Wf + bf            [N, 8]
    g = x @ Wg + bg            [N, 8]
    h = x @ Wh + bh            [N, 64]
    s = f @ g^T                [N, N]
    attn = softmax(s, axis=-1)
    o = gamma * (attn @ h) + x

Kernel strategy (one sample per core):
  - Scores computed TRANSPOSED: sT[m, n] with m (the softmax-reduction index)
    on partitions.  The contraction dim is only K=9 (8 features + affine
    row), so four m-tiles' score matmuls run CONCURRENTLY in the four 32-row
    PE tile_position row groups (f/g both band-replicated across the 4
    bands).  No max subtraction (scores are O(1)); the softmax denominator
    comes free from an augmented column in h.
  - exp split across ScalarE (true exp via activation affine) and VectorE
    (fp8e4m3 Schraudolph bit-trick: i8 = max(s'/16, 0) bitcast to e4m3
    = exp(s)/8), Bresenham-interleaved over [128, 1024] PSUM chunks.
    Scores carry a C1=128*log2(e) scale and +504 offset folded into the
    weights.
  - ctx^T = [128*gamma*h | 128]^T @ exp accumulated in PSUM over m with
    fp8 DoubleRow matmuls (two m-tiles per instruction); row 64 gives
    128*sum(exp), whose reciprocal directly yields gamma*ctx.
  - Epilogue: DMA-transpose ctxT back to [n, c] layout (PE transposes for
    the final quarter), one batched reciprocal per quarter on the DVE, then
    a single fused (ctx*rden + x) scalar_tensor_tensor per n-tile on
    GpSimd so the exp engines stay dedicated to exp.
"""

import numpy as np
import ml_dtypes

import concourse.bass as bass
import concourse.mybir as mybir
import concourse.tile as tile
from concourse.bass import ts, ds
from concourse.bass_utils import run_bass_kernel_spmd
from concourse.masks import make_identity

BF16 = mybir.dt.bfloat16
FP8 = mybir.dt.float8e4
F32 = mybir.dt.float32

N = 4096          # H*W per sample
C = 64            # channels
CK = 8            # f/g projection dim
P = 128           # partitions
NT = N // P       # 32 n/m tiles
HALF = N // 2     # 2048
QW = 1024         # quarter width
NQ = N // QW      # 4
QT = QW // P      # 8 n-tiles per quarter
C1 = 128.0 * np.log2(np.e)   # score pre-scale (f side), undone by ACT affine
SCORE_OFF = 504.0            # additive score offset (exact in bf16):
                             # s'/16 = 8*log2(e)*s + 31.5, the e4m3 bit value
                             # of exp(s)/8 with the Schraudolph shift; clamped
                             # at 0 on the DVE.  ACT computes exp(s - ln 8).
                             # The /8 keeps exp in e4m3 range; softmax ratios
                             # are unaffected.

def _np_bf16(a):
    return np.ascontiguousarray(a.astype(np.float32).astype(ml_dtypes.bfloat16))


def prepare_weights(Wf, bf, Wg, bg, Wh, bh, gamma):
    """Host-side weight folding. Returns dict of bf16 arrays (dram params)."""
    Wf = np.asarray(Wf, np.float32)
    Wg = np.asarray(Wg, np.float32)
    Wh = np.asarray(Wh, np.float32)
    bf = np.asarray(bf, np.float32)
    bg = np.asarray(bg, np.float32)
    bh = np.asarray(bh, np.float32)
    gamma = float(np.asarray(gamma, np.float32))

    # f-side, scaled by C1, bias as row 8 of each band; replicated across
    # the 4 32-row bands so the 4-way row-group-packed score matmuls can
    # contract per band.  Column 32b+8 (paired with g-side column 8 == 1)
    # adds SCORE_OFF to every score so the DVE fp8 bit-trick can clamp at 0
    # instead of going negative: raw scores' = C1*s + SCORE_OFF.
    wf_aug = np.zeros((128, 128), np.float32)
    for b in range(4):
        wf_aug[:C, 32 * b: 32 * b + CK] = C1 * Wf
        wf_aug[C, 32 * b: 32 * b + CK] = C1 * bf
        wf_aug[C, 32 * b + CK] = SCORE_OFF

    # g-side, same replication, column 32b+8 = ones row
    wg_aug = np.zeros((128, 128), np.float32)
    for b in range(4):
        wg_aug[:C, 32 * b: 32 * b + CK] = Wg
        wg_aug[C, 32 * b: 32 * b + CK] = bg
        wg_aug[C, 32 * b + CK] = 1.0

    # h-side scaled by 128*gamma (keeps fp8 h out of subnormals) and a
    # 128-valued denominator column at 64; the epilogue's reciprocal of
    # 128*sum(exp) then yields gamma*ctx directly: [128, 128]
    wh_aug = np.zeros((128, 128), np.float32)
    wh_aug[:C, :C] = 128.0 * gamma * Wh
    wh_aug[C, :C] = 128.0 * gamma * bh
    wh_aug[C, C] = 128.0

    return {
        "wf": _np_bf16(wf_aug),
        "wg": _np_bf16(wg_aug),
        "wh": _np_bf16(wh_aug),
    }


def _spill_excess_waits(nc, limit=1):
    """Walrus rejects HW-queue instructions carrying more than a couple of
    semaphore waits.  Move excess waits onto standalone EventSemaphore
    instructions inserted just before the offender on the same engine
    (cumulative sem-ge waits split across instructions are equivalent)."""
    n_spill = 0
    for bb in nc.main_func.blocks:
        rebuilt = []
        changed = False
        for ins in bb.instructions:
            si = ins.sync_info
            if si is not None and len(si.on_wait) > limit:
                waits = list(si.on_wait)
                for w in waits[limit:]:
                    ev = mybir.InstEventSemaphore(
                        name=f"wspill-{n_spill}", ins=[], outs=[])
                    ev.engine = ins.engine
                    ev.sync_info = mybir.SyncInfo(on_wait=[w], on_update=[])
                    rebuilt.append(ev)
                    n_spill += 1
                ins.sync_info = mybir.SyncInfo(
                    on_wait=waits[:limit], on_update=list(si.on_update))
                changed = True
            rebuilt.append(ins)
        if changed:
            bb.instructions = rebuilt
    return n_spill


def _dedup_ldweights(nc):
    """Drop an InstLdweights whose weight AP/mode is identical to the
    immediately preceding LDW on the PE queue (score j-chunk and DoubleRow
    pairs reuse the same stationary operand).  Only sync-free LDWs are
    dropped so no semaphore edges are lost."""
    n_drop = 0
    for bb in nc.main_func.blocks:
        rebuilt = []
        last_key = None
        changed = False
        for ins in bb.instructions:
            tname = type(ins).__name__
            if tname == "InstLdweights":
                si = ins.sync_info
                clean = si is None or (not si.on_wait and not si.on_update)
                key = (str(ins.ins[0]), str(getattr(ins, "perf_mode", None)),
                       str(getattr(ins, "tile_position", None)),
                       str(getattr(ins, "is_transpose", None)))
                if clean and key == last_key:
                    n_drop += 1
                    changed = True
                    continue
                last_key = key
            elif tname == "InstMatmult":
                pass  # matmul leaves the stationary operand in place
            elif ins.engine == mybir.EngineType.PE:
                last_key = None
            rebuilt.append(ins)
        if changed:
            bb.instructions = rebuilt
    return n_drop


def build_bass(repeat=1, spill=True):
    """Build the per-core Bass graph (SPMD: same graph on all 8 cores).
    repeat > 1 duplicates the whole body for timing calibration."""
    nc = bass.Bass()

    x_d = nc.declare_dram_parameter("x", [P, NT * C], F32, isOutput=False)
    wf_d = nc.declare_dram_parameter("wf", [128, 128], BF16, isOutput=False)
    wg_d = nc.declare_dram_parameter("wg", [128, 128], BF16, isOutput=False)
    wh_d = nc.declare_dram_parameter("wh", [128, 128], BF16, isOutput=False)
    xt_d = nc.declare_dram_parameter("xta", [65, N], BF16, isOutput=False)
    out_d = nc.declare_dram_parameter("out", [N, C], F32, isOutput=True)

    with tile.TileContext(nc) as tc:
        for _ in range(repeat):
            _build_body(nc, tc, x_d, wf_d, wg_d, wh_d, xt_d, out_d)
    _dedup_ldweights(nc)
    if spill:
        _spill_excess_waits(nc)
    return nc


def _build_body(nc, tc, x_d, wf_d, wg_d, wh_d, xt_d, out_d):
    from contextlib import ExitStack

    with ExitStack() as ctx:
        consts = ctx.enter_context(tc.tile_pool(name="consts", bufs=1))
        exp_pool = ctx.enter_context(tc.tile_pool(name="expp", bufs=3))
        work = ctx.enter_context(tc.tile_pool(name="work", bufs=6))

        # ---- prologue DMA: xta + weights first (they gate the projections),
        # residual x tiles behind them.  sync (HWDGE) + gpsimd (SWDGE) queues
        # only: scalar stays free for exp ----
        x_sb = consts.tile([P, NT, C], F32)
        x3 = x_d.rearrange("p (t c) -> p t c", c=C)
        wf_sb = consts.tile([128, 128], BF16)
        wg_sb = consts.tile([128, 128], BF16)
        wh_sb = consts.tile([128, 128], BF16)
        nc.sync.dma_start(wf_sb[:], wf_d[:])
        nc.gpsimd.dma_start(wg_sb[:], wg_d[:])
        nc.sync.dma_start(wh_sb[:], wh_d[:])

        # identity for the final-quarter PE transposes (PE is idle then)
        id_sb = consts.tile([128, 128], BF16)
        make_identity(nc, id_sb[:])

        # --- head warmup: engines are otherwise idle for the NEFF startup +
        # input DMA.  Pull the ScalarE exp table load (~2.7us) and the PE HAM
        # un-throttle (~3.4us of sustained activity) into that window.
        warm = consts.tile([128, 512], BF16)
        nc.vector.memset(warm[:], 0.0)
        wtmp = consts.tile([128, 8], BF16)
        nc.scalar.activation(wtmp[:], warm[:, :8],
                             mybir.ActivationFunctionType.Exp,
                             bias=0.0, scale=1.0)
        with tc.tile_pool(name="warm_ps", bufs=1, space="PSUM") as warm_ps:
            wp = warm_ps.tile([128, 512], F32)
            for _ in range(20):
                nc.tensor.matmul(wp[:], warm[:, :128], warm[:],
                                 start=True, stop=True)

        # ACT exp bias: exp(s'/C1 + bias) = exp(s - ln 8)
        ebias = consts.tile([P, 1], F32)
        nc.vector.memset(ebias[:], float(-SCORE_OFF / C1 - np.log(8.0)))

        # ---- xT_aug [128, N] bf16: rows 0..64 host-built [x^T ; ones],
        # rows 65..127 zeroed on device ----
        xt_sb = consts.tile([128, N], BF16)
        nc.vector.memset(xt_sb[C:, :], 0.0)
        for d in range(2):
            (nc.sync if d == 0 else nc.gpsimd).dma_start(
                xt_sb[:65, ds(d * HALF, HALF)], xt_d[:, ds(d * HALF, HALF)])
        # residual x tiles 0..15 (quarters 0-1), behind xta on the queues
        for d in range(4):
            (nc.sync if d % 2 == 0 else nc.gpsimd).dma_start(
                x_sb[:, ds(4 * d, 4), :], x3[:, ds(4 * d, 4), :])

        with tc.tile_pool(name="pro_ps", bufs=3, space="PSUM") as pro_ps:
            # f/g projections (f scaled by C1), band-replicated.  Emission
            # order front-loads exactly what main-loop group (q0, t) needs:
            # f chunks 0-1 (n 0:1024), then per-t g chunk + h group.
            f_sb = consts.tile([128, N], BF16)
            g_sb = consts.tile([128, N], BF16)
            h_sb = consts.tile([P, NT, 128], FP8)

            def emit_f(chunk):
                pf = pro_ps.tile([128, 512], F32, tag="fg", name="pf")
                nc.tensor.matmul(pf[:], wf_sb[:, :], xt_sb[:, ts(chunk, 512)],
                                 start=True, stop=True)
                nc.any.tensor_copy(f_sb[:, ts(chunk, 512)], pf[:])

            def emit_g(chunk):
                pg = pro_ps.tile([128, 512], F32, tag="fg", name="pg")
                nc.tensor.matmul(pg[:], wg_sb[:, :], xt_sb[:, ts(chunk, 512)],
                                 start=True, stop=True)
                nc.any.tensor_copy(g_sb[:, ts(chunk, 512)], pg[:])

            def emit_h(grp):
                ph = pro_ps.tile([128, 512], F32, tag="fg", name="ph")
                for j in range(4):
                    m = 4 * grp + j
                    nc.tensor.matmul(ph[:, ts(j, P)], xt_sb[:, ts(m, P)],
                                     wh_sb[:], start=True, stop=True)
                nc.any.tensor_copy(h_sb[:, ds(4 * grp, 4), :], ph[:])

            emit_f(0)
            emit_f(1)
            for grp in range(8):
                emit_g(grp)
                emit_h(grp)
            for chunk in range(2, 8):
                emit_f(chunk)

        # x tiles 16..31 (quarters 2-3 residuals, needed late): behind
        # the compute-critical loads on each queue
        for d in range(4, 8):
            (nc.sync if d % 2 == 0 else nc.gpsimd).dma_start(
                x_sb[:, ds(4 * d, 4), :], x3[:, ds(4 * d, 4), :])

        # ---- main: scores -> exp -> ctxT accumulate; epilogue, per quarter.
        # Score matmuls are 4-way row-group packed: band b (rows 32b..32b+31)
        # computes m-tile 4t+b.  Each (q, t) group produces four [128, 2, 512]
        # PSUM pair-tiles (2 banks each; pool of 2 + ctx 2 banks + 1 keep-warm
        # bank = 7 banks).
        #
        # HAM keep-warm: row-group and DoubleRow matmuls do NOT register as
        # PE activity for the HAM clock gate (hardware-measured: the PE
        # re-throttles to 1.2 GHz once only packed/DR matmuls remain), so one
        # small full-array bf16 matmul per group feeds the activity monitor.
        # Its output is allocated from the score pool so pool rotation anchors
        # it to the group cadence (a dep-free dummy gets hoisted to the start
        # by the Tile scheduler).
        with tc.tile_pool(name="ps_s", bufs=3, space="PSUM") as ps_s, \
             tc.tile_pool(name="ps_ctx", bufs=1, space="PSUM") as ps_ctx:
            # exp engine assignment: ACT chunk ~997ns vs DVE ~1192ns -> give
            # ACT ~70 of 128 chunks, spread evenly (Bresenham).
            N_CHUNKS = 128
            DVE_SHARE = 54
            use_dve = [((i * DVE_SHARE) % N_CHUNKS) < DVE_SHARE
                       for i in range(N_CHUNKS)]
            chunk_idx = 0

            def emit_epi_head(q, ctx_ps):
                # epilogue head for quarter q: copy ctxT to SBUF bf16 (split
                # across both exp engines so neither queue blocks long) and
                # kick off the 8 DMA transposes (sync queue, ~1.2us each).
                # Emitted DEFERRED, inside the next quarter's first group.
                ctxt_sb = work.tile([128, QW], BF16, tag="ctxt")
                nc.any.tensor_copy(ctxt_sb[:, :QW // 2], ctx_ps[:, :QW // 2])
                nc.any.tensor_copy(ctxt_sb[:, QW // 2:], ctx_ps[:, QW // 2:])
                o_tr = work.tile([128, QT, P], BF16, tag="otr")
                for t2 in range(QT):
                    nc.sync.dma_start_transpose(
                        o_tr[:, t2, :], ctxt_sb[:, ts(t2, P)])
                return o_tr

            def emit_epi_tile(q, o_tr, t2):
                # one n-tile of quarter q's epilogue: reciprocal of the
                # denominator column, fused (ctx*rden + x), store.  Spread
                # across the NEXT quarter's groups with a 2-group lag so the
                # DVE never waits on an unfinished DMA transpose (a recip
                # that joins the serialized transpose chain blocks all exp
                # chunks queued behind it, stalls the PE, and re-throttles
                # the HAM clock gate).
                blk = o_tr[:, t2, :]
                rden = work.tile([P, 1], F32, tag="rden")
                nc.vector.reciprocal(rden[:], blk[:, C: C + 1])
                osb = work.tile([P, C], F32, tag="osb")
                nc.vector.scalar_tensor_tensor(
                    osb[:], blk[:, :C], rden[:],
                    x_sb[:, q * QT + t2, :],
                    mybir.AluOpType.mult, mybir.AluOpType.add)
                (nc.gpsimd if t2 % 2 == 0 else nc.sync).dma_start(
                    out_d[ds((q * QT + t2) * P, P), :], osb[:])

            prev_ctx = None
            prev_otr = None
            for q in range(NQ):
                ctx_ps = ps_ctx.tile([128, QW], F32, tag="ctx")
                for t in range(8):
                    e_q = exp_pool.tile([128, 4, QW], FP8, tag="e")
                    # Four [128, 2, 512] PSUM pair-tiles per group:
                    #   spF: m-tile 4t+0 via FULL-K matmuls (j0|j1).  The
                    #        band-replicated f/g make full-K contract all 4
                    #        band copies -> 4x the score, undone by a /4 in
                    #        this chunk's exp scale.  These full-array
                    #        matmuls keep the HAM clock gate warm (packed/DR
                    #        matmuls are invisible to it).
                    #   spA/spB: m-tiles 4t+1, 4t+2 row-group packed (bands
                    #        1, 2), one tile per j-chunk.
                    #   spC: m-tile 4t+3 (band 3), j0|j1.
                    spF = ps_s.tile([128, 2, 512], F32, tag="s", name="spF")
                    spA = ps_s.tile([128, 2, 512], F32, tag="s", name="spA")
                    spB = ps_s.tile([128, 2, 512], F32, tag="s", name="spB")
                    spC = ps_s.tile([128, 2, 512], F32, tag="s", name="spC")
                    m0 = 4 * t
                    for j in range(2):
                        nc.tensor.matmul(
                            spF[:, j, :], g_sb[:, ts(m0, P)],
                            f_sb[:, ds(q * QW + j * 512, 512)],
                            start=True, stop=True)
                    for b in (1, 2):
                        for j in range(2):
                            spt = spA if j == 0 else spB
                            nc.tensor.matmul(
                                spt[:, b - 1, :],
                                g_sb[ds(32 * b, 32), ts(m0 + b, P)],
                                f_sb[ds(32 * b, 32), ds(q * QW + j * 512, 512)],
                                start=True, stop=True,
                                tile_position=(32 * b, 0))
                    for j in range(2):
                        nc.tensor.matmul(
                            spC[:, j, :],
                            g_sb[ds(96, 32), ts(m0 + 3, P)],
                            f_sb[ds(96, 32), ds(q * QW + j * 512, 512)],
                            start=True, stop=True,
                            tile_position=(96, 0))
                    # exp: 4 [128, 1024] chunks; spF carries 4x scores
                    for spt, dst, quad in (
                            (spF, e_q[:, ds(0, 1), :], True),
                            (spA, e_q[:, ds(1, 2), ds(0, 512)], False),
                            (spB, e_q[:, ds(1, 2), ds(512, 512)], False),
                            (spC, e_q[:, ds(3, 1), :], False)):
                        scl = 0.25 if quad else 1.0
                        if use_dve[chunk_idx]:
                            i8_view = e_q.bitcast(mybir.dt.int8)
                            i8_dst = (i8_view[:, ds(0, 1), :] if spt is spF
                                      else i8_view[:, ds(1, 2), ds(0, 512)] if spt is spA
                                      else i8_view[:, ds(1, 2), ds(512, 512)] if spt is spB
                                      else i8_view[:, ds(3, 1), :])
                            nc.vector.tensor_scalar(
                                i8_dst, spt[:], scl / 16.0, 0.0,
                                mybir.AluOpType.mult, mybir.AluOpType.max)
                        else:
                            nc.scalar.activation(
                                dst, spt[:],
                                mybir.ActivationFunctionType.Exp,
                                bias=ebias[:], scale=float(scl / C1))
                        chunk_idx += 1
                    # deferred epilogue of the previous quarter: head (copy +
                    # transposes) at t=0 (must precede this quarter's first
                    # ctx matmuls: ps_ctx has bufs=1, so the copies must be
                    # emitted before the bank reuse); tile tails lag 2 groups
                    # behind their transposes: t=2..5 -> tile t-2, t=6 ->
                    # tiles 4,5, t=7 -> tiles 6,7
                    if prev_ctx is not None:
                        if t == 0:
                            prev_otr = emit_epi_head(q - 1, prev_ctx)
                        for t2 in ((t - 2,) if 2 <= t <= 5 else
                                   (4, 5) if t == 6 else
                                   (6, 7) if t == 7 else ()):
                            emit_epi_tile(q - 1, prev_otr, t2)
                    # ctx accumulate: pair-major so each h pair's two j-chunk
                    # matmuls share one DoubleRow LDW (deduped)
                    for pr in range(2):
                        for j in range(2):
                            nc.tensor.matmul(
                                ctx_ps[:, ds(j * 512, 512)],
                                h_sb[:, ds(4 * t + 2 * pr, 2), :],
                                e_q[:, ds(2 * pr, 2), ds(j * 512, 512)],
                                perf_mode=mybir.MatmulPerfMode.DoubleRow,
                                start=(t == 0 and pr == 0),
                                stop=(t == 7 and pr == 1))
                prev_ctx = ctx_ps

            # tail: last quarter's epilogue.  Nothing left for the PE, so
            # transpose on it instead of the DMA xbar (no queue
            # serialization); the DVE is done with exp and reads the PSUM
            # transposes directly.
            ctxt_sb = work.tile([128, QW], BF16, tag="ctxt")
            nc.any.tensor_copy(ctxt_sb[:, :QW // 2], prev_ctx[:, :QW // 2])
            nc.any.tensor_copy(ctxt_sb[:, QW // 2:], prev_ctx[:, QW // 2:])
            tr_ps = ps_s.tile([128, QW], BF16, tag="s", name="trps")
            for t2 in range(QT):
                nc.tensor.transpose(tr_ps[:, ts(t2, P)],
                                    ctxt_sb[:, ts(t2, P)], id_sb[:])
            tr3 = tr_ps.rearrange("p (t c) -> p t c", c=P)
            for t2 in range(QT):
                blk = tr3[:, t2, :]
                rden = work.tile([P, 1], F32, tag="rden")
                nc.vector.reciprocal(rden[:], blk[:, C: C + 1])
                osb = work.tile([P, C], F32, tag="osb")
                nc.vector.scalar_tensor_tensor(
                    osb[:], blk[:, :C], rden[:],
                    x_sb[:, (NQ - 1) * QT + t2, :],
                    mybir.AluOpType.mult, mybir.AluOpType.add)
                (nc.gpsimd if t2 % 2 == 0 else nc.sync).dma_start(
                    out_d[ds(((NQ - 1) * QT + t2) * P, P), :], osb[:])


_CACHE = {}


def _get_nc():
    if "nc" not in _CACHE:
        _CACHE["nc"] = build_bass()
    return _CACHE["nc"]


def kernel(x, Wf, bf, Wg, bg, Wh, bh, gamma):
    x = np.asarray(x, np.float32)
    B = x.shape[0]
    assert x.shape == (B, 64, 64, 64) and B == 8

    w = prepare_weights(Wf, bf, Wg, bg, Wh, bh, gamma)
    nc = _get_nc()
    xt = x.reshape(B, NT, P, C).transpose(0, 2, 1, 3).reshape(B, P, NT * C)
    xta = np.ones((B, 65, N), np.float32)
    xta[:, :C, :] = x.reshape(B, N, C).transpose(0, 2, 1)
    xta = xta.astype(ml_dtypes.bfloat16)
    in_maps = [{"x": np.ascontiguousarray(xt[i]),
                "xta": np.ascontiguousarray(xta[i]), **w} for i in range(B)]
    res = run_bass_kernel_spmd(nc, in_maps, core_ids=list(range(8)))
    out = np.stack([np.asarray(res.results[i]["out"]).reshape(64, 64, 64)
                    for i in range(B)])
    return out.astype(np.float32)
